# revision 2
# baseline (speedup 1.0000x reference)
"""Multi-head causal attention (B=2, S=2048, D=1024, H=16) on 8 trn2 cores.

Sharding: core c handles batch b = c // 4 and head group g = c % 4 (4 heads,
256 feature columns). Each core computes its heads' attention context and a
partial output projection (ctx_g @ Wo[rows_g]); the host sums the 4 partials
per batch and adds bo.

v2 vs the fp32r baseline:
- All matmul operands are bf16 (hosts casts x/W; Q/K/V/P/ctx cast on copy).
  PSUM accumulation stays fp32, softmax denominator stays fp32, output fp32.
  Removes the fp32r narrow-tile 4x penalty and halves input DMA.
- sq-tile-major attention interleaved with the projections and the output
  projection: proj0 proj1 attn0 proj2 attn1 out0 proj3 attn2 out1 attn3
  out2 out3.  Attention for sq-tile n only needs slices <= n, so the PE
  never sees a >3.4us gap (HAM stays at full clock) and the out DMA drains
  early instead of all at the end.
- Within a head, scores matmuls of ski-pair j+1 are emitted before the PV
  matmuls of pair j so the exp (ACT engine) of pair j hides behind PE work.
- kt copies and half the out-proj PSUM drains run on the ACT engine (Copy
  activation, same table set as Exp); the rest on DVE.
- Weight DMAs ordered so the first projection matmul only waits for wq +
  x slice 0; wo streams in mid-kernel.
"""

import os
import sys
import types
from contextlib import ExitStack

import numpy as np
import ml_dtypes

import concourse.bacc as bacc
import concourse.bass as bass
import concourse.mybir as mybir
import concourse.tile as tile
from concourse.bass_utils import run_bass_kernel_spmd


def _install_ntff_hook():
    """The agent image's antenv lacks axon_hooks, so trn_boot's NTFF hook
    install degrades silently. Recreate the module + hook so trace=True works."""
    if "antenv.axon_hooks" in sys.modules:
        return
    try:
        mod = types.ModuleType("antenv.axon_hooks")
        holder = [None]
        mod.set_axon_ntff_profile_hook = lambda h: holder.__setitem__(0, h)
        mod.get_axon_ntff_profile_hook = lambda: holder[0]
        from trn_agent_boot.trn_boot import _ntff_profile_via_ctypes

        hook = _ntff_profile_via_ctypes("/opt/axon/libaxon_pjrt.so")
        if hook is None:
            return
        mod.set_axon_ntff_profile_hook(hook)
        sys.modules["antenv.axon_hooks"] = mod
    except Exception:
        pass

B, S, D, H, HD = 2, 2048, 1024, 16, 64
NCORES = 8
GROUPS = 4          # head groups (cores) per batch
HC = H // GROUPS    # heads per core
DG = HC * HD        # feature columns per core (256)
P = 128
KSUB = D // P       # 8 contraction subtiles for the projections
SQT = 512           # sq tile width (free dim of scores/ctx matmuls)
NSQ = S // SQT      # 4
NST = S // P        # 16 s subtiles of 128
F32 = mybir.dt.float32
BF = mybir.dt.bfloat16
EXP = mybir.ActivationFunctionType.Exp
COPY = mybir.ActivationFunctionType.Copy

_CACHE = {}


def _mha_tile_kernel(tc, xT, wq, wk, wv, wo, out):
    nc = tc.nc
    scale = 1.0 / np.sqrt(np.float32(HD))

    with ExitStack() as ctx:
        consts = ctx.enter_context(tc.tile_pool(name="consts", bufs=1))
        dramp = ctx.enter_context(tc.tile_pool(name="dramp", bufs=3, space="DRAM"))
        # PSUM budget: pps 1-bank x2 + sps 2-bank x2 + cps 1-bank x2 = 8 banks
        pps = ctx.enter_context(tc.tile_pool(name="pps", bufs=2, space="PSUM"))
        sps = ctx.enter_context(tc.tile_pool(name="sps", bufs=2, space="PSUM"))
        cps = ctx.enter_context(tc.tile_pool(name="cps", bufs=2, space="PSUM"))
        xp = ctx.enter_context(tc.tile_pool(name="xp", bufs=3))
        ptp = ctx.enter_context(tc.tile_pool(name="ptp", bufs=3))
        smalls = ctx.enter_context(tc.tile_pool(name="smalls", bufs=3))
        outp = ctx.enter_context(tc.tile_pool(name="outp", bufs=3))

        # --- persistent SBUF tensors ---
        wq_sb = consts.tile([P, KSUB, DG], BF)
        wk_sb = consts.tile([P, KSUB, DG], BF)
        wv_sb = consts.tile([P, KSUB, DG], BF)
        wo_sb = consts.tile([P, DG // P, D], BF)
        qt_sb = consts.tile([P, DG // P, S], BF)   # Q^T: head h at [64*(h%2):, h//2, :]
        # K^T zero-padded per head: head h's 64 rows live at [64*(h%2):, h, :],
        # the other 64 rows are 0 so score matmuls contract over K=128.
        kt_sb = consts.tile([P, HC, S], BF)
        # V with the ones column baked in, per s-subtile and head:
        #   even h: [V(64) | 1 | 0(63)]  -> ctx rows 0-63, denom row 64
        #   odd  h: [1 | 0(63) | V(64)]  -> denom row 0, ctx rows 64-127
        v_sb = consts.tile([P, NST, HC, P], BF)
        ctxt_sb = consts.tile([P, DG // P, S], BF)  # normalized ctx^T, same layout as qt

        xts = []  # per-slice x tiles, filled by emit_xdma

        def emit_xdma(n):
            xn = xp.tile([P, KSUB, SQT], BF, tag="xT", bufs=3, name=f"xn_{n}")
            for k in range(KSUB):
                nc.sync.dma_start(
                    out=xn[:, k, :], in_=xT[k * P : (k + 1) * P, n * SQT : (n + 1) * SQT]
                )
            xts.append(xn)

        # first-needed DMAs first: wq + x slice 0 gate the first matmul
        nc.sync.dma_start(out=wq_sb, in_=wq)
        emit_xdma(0)
        nc.sync.dma_start(out=wk_sb, in_=wk)
        nc.sync.dma_start(out=wv_sb, in_=wv)

        # zero-fill only the kt/v regions the projection copies don't write
        nc.vector.memset(kt_sb[64:P, 0:HC:2, :], 0.0)
        nc.vector.memset(kt_sb[0:64, 1:HC:2, :], 0.0)
        nc.vector.memset(v_sb[:, :, 0:HC:2, HD:P], 0.0)
        nc.vector.memset(v_sb[:, :, 1:HC:2, 0:HD], 0.0)
        for h in range(HC):
            ones_col = 64 if h % 2 == 0 else 0
            nc.vector.memset(v_sb[:, :, h, ones_col : ones_col + 1], 1.0)

        def proj(n):
            """QKV projections for x slice n."""
            xn = xts[n]
            nsl = slice(n * SQT, (n + 1) * SQT)
            for m in range(DG // P):
                ps = pps.tile([P, SQT], F32, tag="p", name=f"qps_{n}_{m}")
                for k in range(KSUB):
                    nc.tensor.matmul(
                        ps,
                        lhsT=wq_sb[:, k, m * P : (m + 1) * P],
                        rhs=xn[:, k, :],
                        start=(k == 0),
                        stop=(k == KSUB - 1),
                    )
                nc.vector.tensor_copy(out=qt_sb[:, m, nsl], in_=ps)
            for m in range(DG // P):
                ps = pps.tile([P, SQT], F32, tag="p", name=f"kps_{n}_{m}")
                for k in range(KSUB):
                    nc.tensor.matmul(
                        ps,
                        lhsT=wk_sb[:, k, m * P : (m + 1) * P],
                        rhs=xn[:, k, :],
                        start=(k == 0),
                        stop=(k == KSUB - 1),
                    )
                # kt copies on ACT engine (DVE is busy with q/v copies)
                nc.scalar.activation(
                    out=kt_sb[0:64, 2 * m, nsl], in_=ps[0:64, :], func=COPY
                )
                nc.scalar.activation(
                    out=kt_sb[64:P, 2 * m + 1, nsl], in_=ps[64:P, :], func=COPY
                )
            st0 = n * (SQT // P)
            for sst in range(SQT // P):
                ps = pps.tile([P, SQT], F32, tag="p", name=f"vps_{n}_{sst}")
                for k in range(KSUB):
                    nc.tensor.matmul(
                        ps[:, 0:DG],
                        lhsT=xn[:, k, sst * P : (sst + 1) * P],
                        rhs=wv_sb[:, k, :],
                        start=(k == 0),
                        stop=(k == KSUB - 1),
                    )
                psv = ps[:, 0:DG].rearrange("p (h d) -> p h d", h=HC, d=HD)
                nc.vector.tensor_copy(
                    out=v_sb[:, st0 + sst, 0:HC:2, 0:HD], in_=psv[:, 0:HC:2, :]
                )
                nc.vector.tensor_copy(
                    out=v_sb[:, st0 + sst, 1:HC:2, HD:P], in_=psv[:, 1:HC:2, :]
                )

        def attn_head(n, h):
            """Scores+softmax+PV for head h, sq-tile n; ski pairs are
            software-pipelined so exp hides behind the next pair's scores."""
            nski = 4 * n + 4
            hm = h // 2
            hp = 64 * (h % 2)
            ctx_rows = 0 if h % 2 == 0 else 64
            denom_row = 64 if h % 2 == 0 else 0
            sq0 = n * SQT
            cpsum = cps.tile([P, SQT], F32, tag="ctx", name=f"ctx_{n}_{h}")
            pend = None

            def emit_pv(infos, pt):
                for ski, w0, base in infos:
                    nc.tensor.matmul(
                        cpsum[:, w0:],
                        lhsT=v_sb[:, ski, h, :],
                        rhs=pt[:, base + w0 : base + SQT],
                        start=(ski == 0),
                        stop=(ski == nski - 1),
                    )

            for j0 in range(0, nski, 2):
                spsum = sps.tile([P, 2 * SQT], F32, tag="s", name=f"s_{n}_{h}_{j0}")
                pt = ptp.tile([P, 2 * SQT], BF, tag="pt", name=f"pt_{n}_{h}_{j0}")
                infos = []
                w0g = None
                for jj in range(2):
                    ski = j0 + jj
                    diag = ski >= 4 * n
                    w0 = (128 * ski - sq0) if diag else 0
                    base = jj * SQT
                    nc.tensor.matmul(
                        spsum[:, base + w0 : base + SQT],
                        lhsT=kt_sb[:, h, ski * P : (ski + 1) * P],
                        rhs=qt_sb[:, hm, sq0 + w0 : sq0 + SQT],
                        start=True,
                        stop=True,
                    )
                    if w0g is None:
                        w0g = base + w0
                    infos.append((ski, w0, base, diag))
                nc.scalar.activation(
                    out=pt[:, w0g:], in_=spsum[:, w0g:],
                    func=EXP, bias=0.0, scale=float(scale),
                )
                for ski, w0, base, diag in infos:
                    if diag:  # zero entries with sk > sq in the triangular block
                        nc.gpsimd.affine_select(
                            out=pt[:, base + w0 : base + w0 + P],
                            in_=pt[:, base + w0 : base + w0 + P],
                            pattern=[[1, P]],
                            compare_op=mybir.AluOpType.is_ge,
                            fill=0.0,
                            base=0,
                            channel_multiplier=-1,
                        )
                if pend is not None:
                    emit_pv(*pend)
                pend = ([i[:3] for i in infos], pt)
            emit_pv(*pend)

            # normalize: ctx rows *= 1/denom (denom broadcast across partitions
            # via a DRAM bounce).
            rec_t = smalls.tile([P, SQT], F32, tag="recip", name=f"rec_{n}_{h}")
            nc.vector.tensor_copy(
                out=rec_t[denom_row : denom_row + 1, :],
                in_=cpsum[denom_row : denom_row + 1, :],
            )
            spread = smalls.tile([P, SQT // P], F32, tag="spread", name=f"spr_{n}_{h}")
            nc.sync.dma_start(out=spread, in_=rec_t[denom_row : denom_row + 1, :])
            nc.vector.reciprocal(out=spread, in_=spread)
            rec_d2 = dramp.tile([1, SQT], F32, tag="rec_d2", name=f"rd_{n}_{h}")
            nc.sync.dma_start(
                out=rec_d2.rearrange("a (p f) -> (a p) f", p=P), in_=spread
            )
            bcast = smalls.tile([P, SQT], F32, tag="bcast", name=f"bc_{n}_{h}")
            rec_b = bass.AP(
                tensor=rec_d2.tensor,
                offset=rec_d2.offset,
                ap=[[0, 64]] + [list(p) for p in rec_d2.ap[1:]],
            )
            nc.sync.dma_start(out=bcast[ctx_rows : ctx_rows + 64, :], in_=rec_b)
            nc.vector.tensor_tensor(
                ctxt_sb[hp : hp + 64, hm, sq0 : sq0 + SQT],
                cpsum[ctx_rows : ctx_rows + 64, :],
                bcast[ctx_rows : ctx_rows + 64, :],
                mybir.AluOpType.mult,
            )

        def attn(n):
            for h in range(HC):
                attn_head(n, h)

        def outproj(n):
            """Partial output projection for st tiles 4n..4n+3."""
            for st in range(4 * n, 4 * n + 4):
                ot = outp.tile([P, D], F32, tag="out", name=f"ot_{st}")
                for nn in range(D // SQT):
                    ps = pps.tile([P, SQT], F32, tag="p", name=f"ops_{st}_{nn}")
                    for k in range(DG // P):
                        nc.tensor.matmul(
                            ps,
                            lhsT=ctxt_sb[:, k, st * P : (st + 1) * P],
                            rhs=wo_sb[:, k, nn * SQT : (nn + 1) * SQT],
                            start=(k == 0),
                            stop=(k == DG // P - 1),
                        )
                    # alternate the PSUM drain between DVE and ACT
                    if nn % 2 == 0:
                        nc.vector.tensor_copy(
                            out=ot[:, nn * SQT : (nn + 1) * SQT], in_=ps
                        )
                    else:
                        nc.scalar.activation(
                            out=ot[:, nn * SQT : (nn + 1) * SQT], in_=ps, func=COPY
                        )
                nc.scalar.dma_start(out=out[st * P : (st + 1) * P, :], in_=ot)

        # --- interleaved schedule ---
        emit_xdma(1)
        proj(0)
        emit_xdma(2)
        proj(1)
        attn(0)
        emit_xdma(3)
        proj(2)
        attn(1)
        nc.sync.dma_start(out=wo_sb, in_=wo)
        outproj(0)
        proj(3)
        attn(2)
        outproj(1)
        attn(3)
        outproj(2)
        outproj(3)


def build_nc():
    if "nc" in _CACHE:
        return _CACHE["nc"]
    nc = bacc.Bacc("TRN2", target_bir_lowering=False, debug=False, num_devices=NCORES)
    xT = nc.dram_tensor("xT", (D, S), BF, kind="ExternalInput").ap()
    wq = nc.dram_tensor("wq", (P, KSUB, DG), BF, kind="ExternalInput").ap()
    wk = nc.dram_tensor("wk", (P, KSUB, DG), BF, kind="ExternalInput").ap()
    wv = nc.dram_tensor("wv", (P, KSUB, DG), BF, kind="ExternalInput").ap()
    wo = nc.dram_tensor("wo", (P, DG // P, D), BF, kind="ExternalInput").ap()
    out = nc.dram_tensor("out", (S, D), F32, kind="ExternalOutput").ap()
    with tile.TileContext(nc) as tc:
        _mha_tile_kernel(tc, xT, wq, wk, wv, wo, out)
    nc.compile()
    _CACHE["nc"] = nc
    return nc


def make_in_maps(x, Wq, Wk, Wv, Wo):
    bf = ml_dtypes.bfloat16
    x = np.asarray(x, np.float32)
    in_maps = []
    for c in range(NCORES):
        b, g = c // GROUPS, c % GROUPS
        cols = slice(g * DG, (g + 1) * DG)

        def wslice(W):
            # [D, DG] -> [128, KSUB, DG] with [p, k, m] = W[k*128+p, m]
            return np.ascontiguousarray(
                np.asarray(W, np.float32)[:, cols]
                .reshape(KSUB, P, DG)
                .transpose(1, 0, 2)
                .astype(bf)
            )

        wo_c = np.ascontiguousarray(
            np.asarray(Wo, np.float32)[cols, :]
            .reshape(DG // P, P, D)
            .transpose(1, 0, 2)
            .astype(bf)
        )
        in_maps.append(
            {
                "xT": np.ascontiguousarray(x[b].T.astype(bf)),
                "wq": wslice(Wq),
                "wk": wslice(Wk),
                "wv": wslice(Wv),
                "wo": wo_c,
            }
        )
    return in_maps


def kernel(x, Wq, Wk, Wv, Wo, bo):
    nc = build_nc()
    in_maps = make_in_maps(x, Wq, Wk, Wv, Wo)
    trace = bool(int(os.environ.get("MHA_TRACE", "0")))
    if trace:
        _install_ntff_hook()
    res = run_bass_kernel_spmd(
        nc, in_maps, core_ids=list(range(NCORES)), trace=trace,
        trace_cores=list(range(NCORES)) if trace else None,
    )
    _CACHE["last_results"] = res
    bo = np.asarray(bo, np.float32)
    out = np.zeros((B, S, D), np.float32)
    for c in range(NCORES):
        out[c // GROUPS] += res.results[c]["out"]
    out += bo[None, None, :]
    return out
